# revision 10
# baseline (speedup 1.0000x reference)
"""CRF forward-algorithm (log partition) kernel for Trainium2, 8 NeuronCores.

Math
----
Reference computes, per batch element b with feats f[b,s,i], transitions
A[i,j], contiguous-prefix mask of length L[b]:

    score_0 = onehot(START) in log space
    score_{s+1}[i] = LSE_j(score_s[j] + A[i,j]) + f[b,s,i]      (while s < L)
    logZ[b] = LSE_i(score_{L}[i] + A[STOP, i])

We run the recurrence in *linear* space:  P_s = exp(score_s) (periodically
rescaled).  One step is  P_{s+1} = (E @ P_s) * ef_s  with E = exp(A) and
ef_s = exp(f[:, s, :]).  Per core (128 batch elements on the free dim,
T=48 tags on the partition dim) each step is ONE TensorE matmul with a
static [48, 96] stationary matrix:

    cols 0..47  : E^T        -> rows 0..47  = E @ P          (the update)
    cols 48..95 : exp(A[STOP,:]) replicated
                 -> rows 48..95 = w = sum_j P[j] * exp(A[STOP,j])
                    (row 48: per-step readout; rows 48..95: broadcast
                     normalizer for the periodic rescale)

followed by ONE VectorE multiply with ef_s.  Every RESC steps P is also
multiplied by 1/w (rows 48..95 give the [48,128] broadcast for free).

Masking never touches the device: masks are contiguous prefixes, so
logZ[b] is just the readout after L[b] steps.  The device stores the raw
readout w_s for every s (plus implicitly the rescale history, which is
the same W rows at the rescale steps), and the host reconstructs

    logZ[b] = log W[L[b], b] + sum_{rescale steps s' < L[b]} log W[s', b]

in float64.
"""

import os
import sys

import numpy as np

for _p in ("/opt/trn_rl_repo",):
    if _p not in sys.path and os.path.isdir(_p):
        sys.path.insert(0, _p)

import ml_dtypes  # noqa: E402

import concourse.bass as bass  # noqa: E402
import concourse.bacc as bacc  # noqa: E402
import concourse.mybir as mybir  # noqa: E402
from concourse import tile  # noqa: E402
from concourse.bass_utils import run_bass_kernel_spmd  # noqa: E402

BF16 = ml_dtypes.bfloat16

B, S, T = 1024, 512, 48
NCORES = 8
BSH = B // NCORES  # 128 batch elements per core
START_IDX, STOP_IDX = 45, 46
RESC = 4  # rescale every RESC steps
CH = 64  # W-readout rows buffered in SBUF before DMA out
NMM = S + 1  # 513 readouts (after 0..512 steps)
RD = 64  # psum partition row holding the w readout (32-aligned for engines)


def build_nc(dtype=mybir.dt.bfloat16, n_steps=S, resc=RESC, ch=CH):
    f32 = mybir.dt.float32
    nmm = n_steps + 1
    assert n_steps % ch == 0
    nc = bacc.Bacc("TRN2", target_bir_lowering=False, debug=False)
    eft_d = nc.declare_dram_parameter("eft", [T, n_steps * BSH], dtype, isOutput=False)
    wmat_d = nc.declare_dram_parameter("wmat", [T, 128], dtype, isOutput=False)
    p0_d = nc.declare_dram_parameter("p0", [T, BSH], dtype, isOutput=False)
    w_d = nc.declare_dram_parameter("W", [nmm, BSH], f32, isOutput=True)

    with tile.TileContext(nc) as tc:
        with (
            tc.tile_pool(name="const", bufs=1) as constp,
            tc.tile_pool(name="eft", bufs=1) as eftp,
            tc.tile_pool(name="state", bufs=4) as statep,
            tc.tile_pool(name="wb", bufs=2) as wbp,
            tc.tile_pool(name="ps", bufs=4, space="PSUM") as psp,
        ):
            wmat_t = constp.tile([T, 128], dtype, tag="wmat")
            nc.sync.dma_start(wmat_t[:], wmat_d[:])

            eft_tiles = []
            for ci in range(n_steps // ch):
                t = eftp.tile([T, ch * BSH], dtype, tag=f"eft{ci}")
                nc.sync.dma_start(
                    t[:], eft_d[:, ci * ch * BSH : (ci + 1) * ch * BSH]
                )
                eft_tiles.append(t)

            p_cur = statep.tile([T, BSH], dtype, tag="pinit")
            nc.sync.dma_start(p_cur[:], p0_d[:])

            wchunk = None
            for s in range(nmm):
                ps_t = psp.tile([128, BSH], f32, tag="ps")
                nc.tensor.matmul(ps_t[:], wmat_t[:], p_cur[:], start=True, stop=True)

                ci, co = divmod(s, ch)
                if co == 0:
                    wchunk = wbp.tile([RD + 1, ch * BSH], f32, tag="wb")
                nc.scalar.activation(
                    wchunk[RD : RD + 1, co * BSH : (co + 1) * BSH],
                    ps_t[RD : RD + 1, :],
                    mybir.ActivationFunctionType.Copy,
                )
                if co == ch - 1 or s == nmm - 1:
                    nc.sync.dma_start(
                        w_d[ci * ch : ci * ch + co + 1, :],
                        wchunk[RD : RD + 1, : (co + 1) * BSH],
                    )
                if s == n_steps:
                    break

                eft_ap = eft_tiles[s // ch][:, (s % ch) * BSH : ((s % ch) + 1) * BSH]
                p_next = statep.tile([T, BSH], dtype, tag="p")
                if s % resc == resc - 1:
                    r_t = statep.tile([T, BSH], f32, tag="r")
                    nc.vector.reciprocal(r_t[:], ps_t[RD : RD + T, :])
                    t1 = statep.tile([T, BSH], f32, tag="t1")
                    nc.vector.tensor_mul(t1[:], ps_t[0:T, :], eft_ap)
                    nc.vector.tensor_mul(p_next[:], t1[:], r_t[:])
                else:
                    nc.vector.tensor_mul(p_next[:], ps_t[0:T, :], eft_ap)
                p_cur = p_next
    nc.compile()
    return nc


def host_prep(feats, transitions):
    """Returns (per-core eft arrays [T, S*BSH] bf16, wmat [T, 2T] bf16)."""
    E = np.exp(transitions.astype(np.float64))  # [T,T], exp(-10000) -> 0
    wmat = np.zeros((T, 128), np.float64)
    wmat[:, :T] = E.T  # wmat[j, i] = E[i, j]  -> psum rows 0..47 = E @ P
    wmat[:, RD : RD + T] = E[STOP_IDX, :][:, None]  # rows 64..111 = w bcast
    wmat_bf = wmat.astype(BF16)

    ef = np.exp(feats.astype(np.float32))  # [B, S, T]
    efts = []
    for c in range(NCORES):
        sl = ef[c * BSH : (c + 1) * BSH]  # [BSH, S, T]
        eft = np.ascontiguousarray(sl.transpose(2, 1, 0))  # [T, S, BSH]
        efts.append(eft.reshape(T, S * BSH).astype(BF16))
    return efts, wmat_bf


def host_finish(W_all, lengths):
    """W_all: [NCORES, NMM, BSH] f32 device readouts. lengths: [B] ints."""
    logW = np.log(W_all.astype(np.float64))  # [NCORES, NMM, BSH]
    resc_rows = logW[:, RESC - 1 : S : RESC, :]  # rescale steps 3,7,...,511
    cum = np.concatenate(
        [np.zeros((NCORES, 1, BSH)), np.cumsum(resc_rows, axis=1)], axis=1
    )  # [NCORES, S//RESC + 1, BSH]
    out = np.empty((B,), np.float32)
    idx = np.arange(BSH)
    for c in range(NCORES):
        Lc = lengths[c * BSH : (c + 1) * BSH]
        out[c * BSH : (c + 1) * BSH] = (
            logW[c, Lc, idx] + cum[c, Lc // RESC, idx]
        ).astype(np.float32)
    return out


def _run(feats, transitions, masks, trace=False):
    feats = np.asarray(feats)
    transitions = np.asarray(transitions)
    masks = np.asarray(masks)
    lengths = masks.sum(axis=1).astype(np.int64)  # [B], in [S//2, S]

    efts, wmat_bf = host_prep(feats, transitions)
    p0 = np.zeros((T, BSH), np.float32)
    p0[START_IDX, :] = 1.0
    p0 = p0.astype(BF16)
    in_maps = [{"eft": efts[c], "wmat": wmat_bf, "p0": p0} for c in range(NCORES)]

    nc = build_nc()
    bres = run_bass_kernel_spmd(
        nc, in_maps, core_ids=list(range(NCORES)), trace=trace
    )
    W_all = np.stack([r["W"] for r in bres.results])  # [NCORES, NMM, BSH]
    return host_finish(W_all, lengths), bres


def kernel(feats, transitions, masks):
    out, _ = _run(feats, transitions, masks, trace=False)
    return out
